# revision 47
# baseline (speedup 1.0000x reference)
"""Trainium2 Bass kernel for AttnBlock (GroupNorm + single-head spatial
self-attention + projection + residual).

Sharding: 8 cores = 4 batches x 2 query-halves. Each core computes
GN + K/V for its batch (duplicated within the pair) and attention +
projection for its half of the 4096 query positions. No collectives.
Host permutes x per core so the query half is always columns [0, NQ).

All matmul operands bf16 (x is shipped bf16; weights pre-cast host-side);
K / VT / Q fully SBUF-resident: no DRAM spill, single attention pass with
PSUM accumulation over all 32 key blocks. The softmax denominator is
reduced across partitions with a ones-matmul on the PE + reciprocal on
DVE so the projection PSUM banks free up ~2us after the last key block
(a gpsimd partition_all_reduce here stalled the PE ~11us per chunk).
A 32KB/partition pad tile keeps attention-phase tiles at SBUF offsets
that avoid PE read-port conflicts (moving them cost ~55ns per matmul).

Math per core (batch b, query half qh, N=4096 keys, NQ=2048 queries):
  h  = groupnorm(x[b])                  [C, N]   bf16
  K  = WkT.T @ h + bk                   [C, N]   bf16
  VT = h.T @ WvT                        [N, C]   bf16 (no bv; folded into bpp)
  Q  = (WqT.T @ hq + bq) * C^-0.5       [C, NQ]  bf16 (scale folded host-side)
  S^T = K.T @ Q -> E = exp(S^T)         [N, NQ]  (no max-sub; scores are O(5))
  O  = VT.T @ E (unnormalized)          [C, NQ];  den = sum_j E
  out = xq + (WpT.T @ O) / den + bpp    where bpp = Wp@bv + bp
"""
import math
import numpy as np

import concourse.bass as bass
import concourse.bacc as bacc
import concourse.tile as tile
from concourse import mybir
from concourse.bass_utils import run_bass_kernel_spmd

F32 = mybir.dt.float32
F32R = mybir.dt.float32r
BF16 = mybir.dt.bfloat16
AF = mybir.ActivationFunctionType
ALU = mybir.AluOpType

C = 512          # channels
N = 4096         # spatial positions (keys)
NQ = 2048        # queries per core
CT = 4           # channel tiles of 128
ICN = 4          # i-chunks per core
ICW = 512        # i-chunk width
JBN = 32         # j-blocks (128 wide)
JCN = 8          # j 512-chunks
GROUPS = 32
EPS = 1e-6
INV = 1.0 / math.sqrt(C)
BN_FMAX = 512
HW_ = 2048       # x/h stored as [t][half] tiles of this width


def _emit(nc, tc, ctx, tens, rep, qh=0):
    r = f"r{rep}_"
    XF = tens["XF"]
    WQT, WKT, WVT, WPT = tens["WQT"], tens["WKT"], tens["WVT"], tens["WPT"]
    GM = tens["GM"]
    OUT = tens["OUT"]

    const = ctx.enter_context(tc.tile_pool(name=r + "const", bufs=1))
    wpool = ctx.enter_context(tc.tile_pool(name=r + "wp", bufs=1))
    hpool = ctx.enter_context(tc.tile_pool(name=r + "hp", bufs=1))
    xqp = ctx.enter_context(tc.tile_pool(name=r + "xq", bufs=1))
    xpool = tc.alloc_tile_pool(name=r + "xp", bufs=1)

    # x tiles: [t][half] of [128, 2048] bf16; query half reused as residual.
    # xpool holds a 32KB/partition pad tile released where the f32 x halves
    # used to be, keeping downstream pools at the same SBUF offsets.
    x_t = [[xqp.tile([128, HW_], BF16, name=f"{r}x{t}_{hh}", tag=f"x{t}_{hh}")
            for hh in range(2)] for t in range(CT)]
    xpool.tile([128, 16384], BF16, name=r + "padt")

    def _load_x(t):
        for hh in range(2):
            nc.sync.dma_start(
                out=x_t[t][hh],
                in_=XF[t * 128:(t + 1) * 128, hh * HW_:(hh + 1) * HW_])

    gm_t = const.tile([128, 128], F32, name=r + "gm")
    cvec = const.tile([128, 20], F32, name=r + "cvec")
    bq_t = [cvec[:, cb:cb + 1] for cb in range(CT)]
    bk_t = [cvec[:, 4 + cb:5 + cb] for cb in range(CT)]
    bpp_t = [cvec[:, 8 + cb:9 + cb] for cb in range(CT)]
    gns_t = [cvec[:, 12 + t:13 + t] for t in range(CT)]
    gnb_t = [cvec[:, 16 + t:17 + t] for t in range(CT)]
    eps_t = const.tile([128, 1], F32, name=r + "eps")
    nc.vector.memset(eps_t, EPS)
    ones_f = const.tile([128, 128], F32, name=r + "onesf")
    nc.vector.memset(ones_f, 1.0)
    ones_t = const.tile([128, 128], F32R, name=r + "ones")
    nc.vector.tensor_copy(out=ones_t, in_=ones_f)
    a_t = [const.tile([128, 1], F32, name=f"{r}a{t}", tag=f"a{t}") for t in range(CT)]
    c2_t = [const.tile([128, 1], F32, name=f"{r}c2{t}", tag=f"c2{t}") for t in range(CT)]

    wq_a = wpool.tile([128, CT * C], BF16, name=f"{r}wq", tag="wq")
    wk_a = wpool.tile([128, CT * C], BF16, name=f"{r}wk", tag="wk")
    wv_a = wpool.tile([128, CT * C], BF16, name=f"{r}wv", tag="wv")
    wp_a = wpool.tile([128, CT * C], BF16, name=f"{r}wp", tag="wp")
    wq_t = [wq_a[:, t * C:(t + 1) * C] for t in range(CT)]
    wk_t = [wk_a[:, t * C:(t + 1) * C] for t in range(CT)]
    wv_t = [wv_a[:, t * C:(t + 1) * C] for t in range(CT)]
    wp_t = [wp_a[:, t * C:(t + 1) * C] for t in range(CT)]
    _load_x(0)
    nc.sync.dma_start(out=cvec, in_=tens["CVEC"][:, :])
    nc.sync.dma_start(out=gm_t, in_=GM[:, :])
    nc.gpsimd.dma_start(out=wv_a, in_=WVT[:, :])
    _load_x(1)
    nc.gpsimd.dma_start(out=wk_a, in_=WKT[:, :])
    _load_x(2)
    nc.gpsimd.dma_start(out=wq_a, in_=WQT[:, :])
    _load_x(3)
    nc.gpsimd.dma_start(out=wp_a, in_=WPT[:, :])

    # h tiles: [t][half] of [128, 2048] bf16
    h_t = [[hpool.tile([128, HW_], BF16, name=f"{r}h{t}_{hh}", tag=f"h{t}_{hh}")
            for hh in range(2)] for t in range(CT)]

    def hsl(t, col, w):
        """slice of h for channel-tile t covering [col, col+w) of N."""
        hh, off = col // HW_, col % HW_
        return h_t[t][hh][:, off:off + w]

    # ================= PHASE 1: GN -> h =================
    with (
        tc.tile_pool(name=r + "pgn", bufs=2) as pgn,
        tc.tile_pool(name=r + "gps", bufs=2, space="PSUM") as gps,
    ):
        for t in range(CT):
            stats = pgn.tile([128, 8, 6], F32, name=f"{r}st{t}", tag="stats")
            for hh in range(2):
                for s in range(4):
                    nc.vector.bn_stats(
                        out=stats[:, 4 * hh + s, :],
                        in_=x_t[t][hh][:, s * BN_FMAX:(s + 1) * BN_FMAX])
            mv = pgn.tile([128, 2], F32, name=f"{r}mv{t}", tag="mv")
            nc.vector.bn_aggr(out=mv, in_=stats)
            # t2 = [mean, var + mean^2]
            t2 = pgn.tile([128, 2], F32, name=f"{r}t2{t}", tag="t2")
            nc.vector.tensor_copy(out=t2[:, 0:1], in_=mv[:, 0:1])
            sq = pgn.tile([128, 1], F32, name=f"{r}sq{t}", tag="sq")
            nc.vector.tensor_mul(out=sq, in0=mv[:, 0:1], in1=mv[:, 0:1])
            nc.vector.tensor_add(out=t2[:, 1:2], in0=mv[:, 1:2], in1=sq)
            chp = gps.tile([128, 2], F32, name=f"{r}chp{t}", tag="gp")
            nc.tensor.matmul(chp, gm_t, t2, start=True, stop=True)
            ch = pgn.tile([128, 2], F32, name=f"{r}ch{t}", tag="ch")
            nc.vector.tensor_copy(out=ch, in_=chp)
            gmean, gmsq = ch[:, 0:1], ch[:, 1:2]
            sg = pgn.tile([128, 1], F32, name=f"{r}sg{t}", tag="sg")
            nc.vector.tensor_mul(out=sg, in0=gmean, in1=gmean)
            gv = pgn.tile([128, 1], F32, name=f"{r}gv{t}", tag="gv")
            nc.vector.tensor_sub(out=gv, in0=gmsq, in1=sg)
            nc.scalar.activation(out=gv, in_=gv, func=AF.Sqrt, bias=eps_t, scale=1.0)
            nc.vector.reciprocal(out=gv, in_=gv)
            nc.vector.tensor_mul(out=a_t[t], in0=gv, in1=gns_t[t])
            tmp = pgn.tile([128, 1], F32, name=f"{r}tm{t}", tag="tm")
            nc.vector.tensor_mul(out=tmp, in0=gmean, in1=a_t[t])
            nc.vector.tensor_sub(out=c2_t[t], in0=gnb_t[t], in1=tmp)

            # h = x * a + c2 (bf16): query half on ACT now; the other
            # half (consumed ~14us after PE start by VT jb16+) is
            # deferred for t0-t2 so it doesn't sit in the DVE queue
            # ahead of the last tile's stats, which gate PE ramp-up.
            nc.scalar.activation(
                out=h_t[t][qh], in_=x_t[t][qh],
                func=AF.Identity, bias=c2_t[t], scale=a_t[t])
            if t == CT - 1:
                nc.vector.tensor_scalar(
                    out=h_t[t][1 - qh], in0=x_t[t][1 - qh],
                    scalar1=a_t[t], scalar2=c2_t[t], op0=ALU.mult,
                    op1=ALU.add)
                for td in range(CT - 1):
                    nc.vector.tensor_scalar(
                        out=h_t[td][1 - qh], in0=x_t[td][1 - qh],
                        scalar1=a_t[td], scalar2=c2_t[td], op0=ALU.mult,
                        op1=ALU.add)

    xpool.release()

    # ================= PHASE 1b: VT, K, Q =================
    kpool = ctx.enter_context(tc.tile_pool(name=r + "kres", bufs=1))
    vpool = ctx.enter_context(tc.tile_pool(name=r + "vres", bufs=1))
    qpool = ctx.enter_context(tc.tile_pool(name=r + "qres", bufs=1))
    k_sb = [[kpool.tile([128, 512], BF16, name=f"{r}k{cb}_{jc}", tag=f"k{cb}_{jc}")
             for jc in range(JCN)] for cb in range(CT)]
    vt_sb = [vpool.tile([128, 512], BF16, name=f"{r}vt{jb}", tag=f"vt{jb}")
             for jb in range(JBN)]
    q_sb = [qpool.tile([128, NQ], BF16, name=f"{r}q{t}", tag=f"q{t}") for t in range(CT)]

    with tc.tile_pool(name=r + "pps1", bufs=6, space="PSUM") as pps1:
        # --- VT = h.T @ WvT : [N, C] ---
        for jb in range(JBN):
            vp = pps1.tile([128, 512], F32, name=f"{r}vp{jb}", tag="mm")
            for t in range(CT):
                nc.tensor.matmul(vp, hsl(t, jb * 128, 128), wv_t[t],
                                 start=(t == 0), stop=(t == CT - 1))
            nc.scalar.copy(out=vt_sb[jb], in_=vp)

        # --- K = WkT.T @ h + bk : [C, N] ---
        for cb in range(CT):
            for jc in range(JCN):
                kp = pps1.tile([128, 512], F32, name=f"{r}kp{cb}_{jc}", tag="mm")
                for t in range(CT):
                    nc.tensor.matmul(kp, wk_t[t][:, cb * 128:(cb + 1) * 128],
                                     hsl(t, jc * 512, 512),
                                     start=(t == 0), stop=(t == CT - 1))
                nc.vector.tensor_scalar(out=k_sb[cb][jc], in0=kp,
                                        scalar1=bk_t[cb], scalar2=None,
                                        op0=ALU.add, op1=ALU.bypass)

        # --- Q = WqT.T @ hq + bq (pre-scaled) : [C, NQ] ---
        for icc in range(ICN):
            for cb in range(CT):
                qp = pps1.tile([128, ICW], F32, name=f"{r}qp{cb}_{icc}", tag="mm")
                for t in range(CT):
                    nc.tensor.matmul(qp, wq_t[t][:, cb * 128:(cb + 1) * 128],
                                     h_t[t][qh][:, icc * ICW:(icc + 1) * ICW],
                                     start=(t == 0), stop=(t == CT - 1))
                nc.scalar.add(out=q_sb[cb][:, icc * ICW:(icc + 1) * ICW], in_=qp,
                              add=bq_t[cb])

    # ================= PHASE 2: attention =================
    pdenp = ctx.enter_context(tc.tile_pool(name=r + "pden", bufs=1))
    part_den = [pdenp.tile([128, ICW], F32R, name=f"{r}pd{ic}", tag=f"pd{ic}")
                for ic in range(ICN)]
    ep = ctx.enter_context(tc.tile_pool(name=r + "ep", bufs=4))
    fin = ctx.enter_context(tc.tile_pool(name=r + "fin", bufs=2))
    op = ctx.enter_context(tc.tile_pool(name=r + "op", bufs=1, space="PSUM"))
    pps2 = ctx.enter_context(tc.tile_pool(name=r + "pps2", bufs=4, space="PSUM"))

    # Software pipeline: each chunk's finalize is emitted after the NEXT
    # chunk's first two score-groups, so the PE chews on st(ic+1) while the
    # den tail (last exp -> last add -> ones-matmul -> reciprocal -> t1)
    # resolves and frees the proj PSUM banks.
    def emit_st(ic, jb):
        st = pps2.tile([128, ICW], F32, name=f"{r}s{ic}_{jb}", tag="mm")
        for cb in range(CT):
            nc.tensor.matmul(
                st, k_sb[cb][jb // 4][:, (jb % 4) * 128:(jb % 4 + 1) * 128],
                q_sb[cb][:, ic * ICW:(ic + 1) * ICW],
                start=(cb == 0), stop=(cb == CT - 1))
        e = ep.tile([128, ICW], BF16, name=f"{r}e{ic}_{jb}", tag="e")
        nc.scalar.activation(out=e, in_=st, func=AF.Exp, scale=1.0)
        deng = nc.vector if jb % 2 == 0 else nc.gpsimd
        if jb == 0:
            deng.tensor_copy(out=part_den[ic], in_=e)
        else:
            deng.tensor_add(out=part_den[ic], in0=part_den[ic], in1=e)
        return e

    def emit_finalize(ic, o_ps):
        # o_f copies first so the DVE-side ones aren't queued behind the
        # 3.4us reciprocal; the reciprocal then overlaps the proj matmuls.
        o_f = []
        for cb in range(CT):
            of = ep.tile([128, ICW], BF16, name=f"{r}of{cb}_{ic}", tag=f"of{cb}",
                         bufs=1)
            if cb % 2 == 0:
                nc.scalar.copy(out=of, in_=o_ps[cb])
            else:
                nc.vector.tensor_copy(out=of, in_=o_ps[cb])
            o_f.append(of)
        dn = pps2.tile([128, ICW], F32, name=f"{r}dn{ic}", tag="mm")
        nc.tensor.matmul(dn, ones_t, part_den[ic], start=True, stop=True)
        rb = ep.tile([128, ICW], F32, name=f"{r}rb{ic}", tag="rb", bufs=2)
        nc.vector.reciprocal(out=rb, in_=dn)
        xbs = []
        for cb in range(CT):
            # xb = x + bpp on ACT, off the critical path
            xb = fin.tile([128, ICW], F32, name=f"{r}xb{cb}_{ic}", tag="xb",
                          bufs=4)
            nc.scalar.add(out=xb, in_=x_t[cb][qh][:, ic * ICW:(ic + 1) * ICW],
                          add=bpp_t[cb])
            xbs.append(xb)
        for cb in range(CT):
            pp = pps2.tile([128, ICW], F32, name=f"{r}p{cb}_{ic}", tag="mm")
            for t in range(CT):
                nc.tensor.matmul(pp, wp_t[t][:, cb * 128:(cb + 1) * 128],
                                 o_f[t], start=(t == 0), stop=(t == CT - 1))
            t1 = fin.tile([128, ICW], F32, name=f"{r}t1{cb}_{ic}", tag="t1")
            nc.vector.tensor_mul(out=t1, in0=pp, in1=rb)
            ot = fin.tile([128, ICW], F32, name=f"{r}ot{cb}_{ic}", tag="ot")
            nc.vector.tensor_add(out=ot, in0=t1, in1=xbs[cb])
            nc.sync.dma_start(
                out=OUT[cb * 128:(cb + 1) * 128, ic * ICW:(ic + 1) * ICW],
                in_=ot)

    pending = None  # (ic, o_ps) finalize deferred past next chunk's prologue
    for ic in range(ICN):
        o_ps = [op.tile([128, ICW], F32, name=f"{r}o{cb}_{ic}", tag=f"o{cb}")
                for cb in range(CT)]
        es = [emit_st(ic, 0), emit_st(ic, 1)]
        if pending is not None:
            emit_finalize(*pending)
        for jb in range(JBN):
            e = es[jb]
            for cb in range(CT):
                nc.tensor.matmul(o_ps[cb], vt_sb[jb][:, cb * 128:(cb + 1) * 128],
                                 e, start=(jb == 0), stop=(jb == JBN - 1))
            if jb + 2 < JBN:
                es.append(emit_st(ic, jb + 2))
        pending = (ic, o_ps)
    emit_finalize(*pending)


def _build(reps=1):
    nc = bacc.Bacc()
    tens = {
        "XF": nc.dram_tensor("XF", [C, N], BF16, kind="ExternalInput"),
        "WQT": nc.dram_tensor("WQT", [128, CT * C], BF16, kind="ExternalInput"),
        "WKT": nc.dram_tensor("WKT", [128, CT * C], BF16, kind="ExternalInput"),
        "WVT": nc.dram_tensor("WVT", [128, CT * C], BF16, kind="ExternalInput"),
        "WPT": nc.dram_tensor("WPT", [128, CT * C], BF16, kind="ExternalInput"),
        "CVEC": nc.dram_tensor("CVEC", [128, 20], F32, kind="ExternalInput"),
        "GM": nc.dram_tensor("GM", [128, 128], F32, kind="ExternalInput"),
        "OUT": nc.dram_tensor("OUT", [C, NQ], F32, kind="ExternalOutput"),
    }
    with tile.TileContext(nc) as tc:
        from contextlib import ExitStack as ES
        for rep in range(reps):
            with ES() as ctx:
                _emit(nc, tc, ctx, tens, rep)
    nc.finalize()
    return nc


_NC_CACHE = {}


def _get_nc(reps=1):
    if reps not in _NC_CACHE:
        _NC_CACHE[reps] = _build(reps)
    return _NC_CACHE[reps]


def _prep_inputs(x, gn_scale, gn_bias, wq, bq, wk, bk, wv, bv, wp, bp):
    import ml_dtypes
    bf16 = ml_dtypes.bfloat16
    x = np.ascontiguousarray(np.asarray(x, dtype=np.float32))
    B = x.shape[0]
    xb = x.reshape(B, C, N).astype(bf16)
    f32 = lambda v: np.ascontiguousarray(np.asarray(v, dtype=np.float32))
    wq, wk, wv, wp = f32(wq), f32(wk), f32(wv), f32(wp)
    bq, bk, bv, bp = f32(bq), f32(bk), f32(bv), f32(bp)
    wbf = lambda w: np.ascontiguousarray(
        w.T.reshape(CT, 128, C).transpose(1, 0, 2).reshape(128, CT * C)
        .astype(bf16))
    common = {
        "WQT": wbf(wq * INV),
        "WKT": wbf(wk),
        "WVT": wbf(wv),
        "WPT": wbf(wp),
        "CVEC": np.ascontiguousarray(np.concatenate(
            [v.reshape(CT, 128).T for v in
             [bq * INV, bk, (wp @ bv + bp).astype(np.float32),
              f32(gn_scale), f32(gn_bias)]], axis=1), dtype=np.float32),
        "GM": np.kron(np.eye(8, dtype=np.float32),
                      np.full((16, 16), 1.0 / 16.0, np.float32)),
    }
    in_maps = []
    for core in range(8):
        b, h = core // 2, core % 2
        m = dict(common)
        # permute so this core's query half is always columns [0, NQ)
        m["XF"] = (xb[b] if h == 0 else np.ascontiguousarray(
            np.concatenate([xb[b][:, NQ:], xb[b][:, :NQ]], axis=1)))
        in_maps.append(m)
    return in_maps, B


def kernel(**inputs):
    nc = _get_nc(1)
    in_maps, B = _prep_inputs(**inputs)
    res = run_bass_kernel_spmd(nc, in_maps, core_ids=list(range(8)))
    out = np.empty((B, C, N), dtype=np.float32)
    for core in range(8):
        b, h = core // 2, core % 2
        out[b][:, h * NQ:(h + 1) * NQ] = res.results[core]["OUT"]
    return out.reshape(B, C, 64, 64)
